# revision 39
# baseline (speedup 1.0000x reference)
"""MultiHeadCoupledAttention Trainium2 kernel (8 NeuronCores).

Sharding: core c handles batch b = c//4 and heads 4g..4g+3 where g = c%4
(data parallel on B, tensor parallel on heads / d_k; out_proj row-parallel
with the partial sums reduced on host).

Per-core device pipeline (all matmuls bf16, PSUM f32):
  1. QK projections (transposed layout [d_k, L]) evicted to bf16 on
     ScalarE, RoPE on VectorE with an in-SBUF half-swap DMA.
  2. Per (head, l-half): scores S^T[s,l] via K=64 matmuls; silu on
     ScalarE straight out of PSUM; add on VectorE; one big exp
     (scale=1/sqrt2) on ScalarE.  l-halving keeps the exp buffer
     double-buffered inside the SBUF budget.
  3. PV matmul with a ones-column appended to V: rows 0..63 = exp @ v,
     row 64 = softmax denominator.
  4. attn output: XBAR DMA-transpose of the unnormalized exp to [l,s],
     per-partition tensor_scalar normalize (reciprocal transposed via a
     second tiny XBAR transpose), casting SWDGE DMA store (bf16 -> f32).
  5. out_proj: OT_norm^T @ Wout_slice^T -> partial [L, 1024] f32.
"""

import math

import numpy as np
import ml_dtypes

import concourse.bass as bass
import concourse.mybir as mybir
import concourse.tile as tile
from concourse import bacc
from concourse.bass_utils import run_bass_kernel_spmd

BF16 = mybir.dt.bfloat16
F32 = mybir.dt.float32
AF = mybir.ActivationFunctionType
OP = mybir.AluOpType

P = 128
L = 2048          # sequence length
LH = 1024         # l-half width
DM = 1024         # d_model
NH = 4            # heads per core
HD = 64           # head dim
DK = NH * HD      # per-core d_k slice (256)
N_CORES = 8
SC = L // P       # 16 s-chunks

INV_SQRT_D = 1.0 / math.sqrt(HD)
INV_SQRT_2 = 1.0 / math.sqrt(2.0)

# exp(u/sqrt2) ~ C * ((u+C1)^2 + D1) * ((u+C2)^2 + D2) on u in [-0.45, 2.2]
# (C cancels in softmax).  Constants fitted against the exact bf16
# evaluation chain; max rel err ~0.97% incl. rounding.
EC1 = 1.58173174
ED1 = 1.27719784
EC2 = -0.69679504
ED2 = 10.81390516

_CACHED = {}


def _build_nc():
    nc = bacc.Bacc("TRN2", target_bir_lowering=False, debug=False)

    qT_d = nc.declare_dram_parameter("qT", [DM, L], BF16, isOutput=False)
    w_d = nc.declare_dram_parameter("w", [DM, 4, DK], BF16, isOutput=False)
    vo_d = nc.declare_dram_parameter("vo", [L, NH, HD + 1], BF16, isOutput=False)
    woutT_d = nc.declare_dram_parameter("woutT", [DK, DM], BF16, isOutput=False)
    cos_d = nc.declare_dram_parameter("cosT", [P, L], BF16, isOutput=False)
    sin_d = nc.declare_dram_parameter("sinT", [P, L], BF16, isOutput=False)
    attn_d = nc.declare_dram_parameter("attn", [NH, L, L], F32, isOutput=True)
    part_d = nc.declare_dram_parameter("part", [L, DM], F32, isOutput=True)

    with tile.TileContext(nc) as tc:
        with tc.tile_pool(name="const", bufs=1) as c0, \
             tc.tile_pool(name="proj", bufs=1) as projp:

            vo_sb = c0.tile([P, SC, NH, HD + 1], BF16)
            woutT_sb = c0.tile([P, 2, DM], BF16)

            # ---------------- Phase 1: projections + RoPE ----------------
            # QK[t] layout: [P, 2, L], dk row = dc*128 + p, t in (qw, kw, qp, kp)
            QK = [projp.tile([P, 2, L], BF16, name=f"qk{t}") for t in range(4)]
            with tc.tile_pool(name="ph1", bufs=1) as ph1, \
                 tc.tile_pool(name="ph1t", bufs=2) as ph1t, \
                 tc.tile_pool(name="ph1p", bufs=4, space="PSUM") as ph1p:
                qT_sb = ph1.tile([P, 8, L], BF16)
                w_sb = ph1.tile([P, 8, 4, DK], BF16)
                # chunked loads so the first projection matmuls start early
                for k in range(8):
                    nc.sync.dma_start(
                        qT_sb[:, k, :],
                        qT_d[:].rearrange("(ko p) l -> p ko l", p=P)[:, k, :])
                    nc.sync.dma_start(
                        w_sb[:, k, :, :],
                        w_d[:].rearrange("(ko p) t m -> p ko t m", p=P)[:, k, :, :])
                cos_sb = ph1.tile([P, L], BF16)
                nc.sync.dma_start(cos_sb[:], cos_d[:])
                sin_sb = ph1.tile([P, L], BF16)
                nc.sync.dma_start(sin_sb[:], sin_d[:])
                nc.sync.dma_start(
                    vo_sb[:], vo_d[:].rearrange("(sc p) h m -> p sc h m", p=P))
                nc.sync.dma_start(
                    woutT_sb[:], woutT_d[:].rearrange("(dc p) m -> p dc m", p=P))

                # dc-major so heads 0/1 (dc=0) unblock before dc=1 finishes;
                # qw/kp first so the w2p score matmuls can start earliest
                for dc in range(2):
                    for t in (0, 3, 1, 2):
                        xev = ph1t.tile([P, L], BF16, tag="xev")
                        for lc in range(4):
                            ps = ph1p.tile([P, 512], F32, tag="pp")
                            for k in range(8):
                                nc.tensor.matmul(
                                    ps[:],
                                    lhsT=w_sb[:, k, t, dc * 128:(dc + 1) * 128],
                                    rhs=qT_sb[:, k, lc * 512:(lc + 1) * 512],
                                    start=(k == 0), stop=(k == 7),
                                )
                            nc.scalar.activation(
                                xev[:, lc * 512:(lc + 1) * 512], ps[:], AF.Copy)
                        tcb = ph1t.tile([P, L], BF16, tag="tcb")
                        nc.vector.tensor_tensor(tcb[:], xev[:], cos_sb[:], OP.mult)
                        tsb = ph1t.tile([P, L], BF16, tag="tsb")
                        nc.vector.tensor_tensor(tsb[:], xev[:], sin_sb[:], OP.mult)
                        tswb = ph1t.tile([P, L], BF16, tag="tswb")
                        nc.sync.dma_start(tswb[0:32, :], tsb[32:64, :])
                        nc.sync.dma_start(tswb[32:64, :], tsb[0:32, :])
                        nc.sync.dma_start(tswb[64:96, :], tsb[96:128, :])
                        nc.sync.dma_start(tswb[96:128, :], tsb[64:96, :])
                        nc.vector.tensor_tensor(QK[t][:, dc, :], tcb[:], tswb[:], OP.add)

            OT_all = projp.tile([P, 2, L], BF16, name="ot_all")

            # ---------------- Phase 2: per (head, l-half) attention ----------------
            with tc.tile_pool(name="big", bufs=2) as bigp, \
                 tc.tile_pool(name="sct", bufs=4) as sct, \
                 tc.tile_pool(name="tbp", bufs=1) as tbp, \
                 tc.tile_pool(name="smal", bufs=2) as smal, \
                 tc.tile_pool(name="ps2", bufs=3, space="PSUM") as ps2, \
                 tc.tile_pool(name="pspv", bufs=2, space="PSUM") as pspv:

                def emit_outproj(lc_lo, lc_hi):
                    for lc in range(lc_lo, lc_hi):
                        po = ps2.tile([P, 1024], F32, tag="scps")
                        for ns in range(2):
                            for dcq in range(2):
                                nc.tensor.matmul(
                                    po[:, ns * 512:(ns + 1) * 512],
                                    lhsT=OT_all[:, dcq, lc * P:(lc + 1) * P],
                                    rhs=woutT_sb[:, dcq, ns * 512:(ns + 1) * 512],
                                    start=(dcq == 0), stop=(dcq == 1),
                                )
                        osb = smal.tile([P, 1024], F32, tag="osb")
                        nc.any.tensor_copy(osb[:], po[:])
                        nc.sync.dma_start(part_d[lc * P:(lc + 1) * P, :], osb[:])

                for h in range(NH):
                    for half in range(2):
                        dc = h // 2
                        bp = 64 * (h % 2)
                        kpT = QK[3][bp:bp + 64, dc, :]
                        qwT = QK[0][bp:bp + 64, dc, :]
                        kwT = QK[1][bp:bp + 64, dc, :]
                        qpT = QK[2][bp:bp + 64, dc, :]
                        l0 = half * LH
                        expT = bigp.tile([P, SC, LH], BF16, tag="expT")
                        # ScalarE is the kernel's critical engine, so most
                        # halves evaluate exp as a factored quartic on
                        # VectorE (interleaved with the score adds so the
                        # DVE FIFO never starves ScalarE); a few halves stay
                        # on ScalarE to balance the two engines.
                        act_exp = True  # poly-exp path kept but disabled: scheduler loses more than engine balance gains

                        for sc in range(SC):
                            ssl = slice(sc * P, (sc + 1) * P)
                            s1 = sct.tile([P, LH], BF16, tag="s1")
                            s2 = sct.tile([P, LH], BF16, tag="s2")
                            psA = ps2.tile([P, LH], F32, tag="scps")
                            for lq in range(2):
                                q0 = l0 + lq * 512
                                nc.tensor.matmul(
                                    psA[:, lq * 512:(lq + 1) * 512],
                                    lhsT=kpT[:, ssl], rhs=qwT[:, q0:q0 + 512],
                                    start=True, stop=True,
                                )
                            nc.scalar.activation(s1[:], psA[:], AF.Silu, scale=INV_SQRT_D)
                            psB = ps2.tile([P, LH], F32, tag="scps")
                            for lq in range(2):
                                q0 = l0 + lq * 512
                                nc.tensor.matmul(
                                    psB[:, lq * 512:(lq + 1) * 512],
                                    lhsT=kwT[:, ssl], rhs=qpT[:, q0:q0 + 512],
                                    start=True, stop=True,
                                )
                            nc.scalar.activation(s2[:], psB[:], AF.Silu, scale=INV_SQRT_D)
                            nc.vector.tensor_tensor(expT[:, sc, :], s1[:], s2[:], OP.add)
                            if not act_exp and sc % 2 == 1:
                                X = expT[:, sc - 1:sc + 1, :]
                                pq1 = sct.tile([P, 2, LH], BF16, tag="pq1")
                                nc.vector.tensor_scalar_add(pq1[:], X, EC1)
                                nc.vector.tensor_tensor(pq1[:], pq1[:], pq1[:], OP.mult)
                                nc.vector.tensor_scalar_add(pq1[:], pq1[:], ED1)
                                pq2 = sct.tile([P, 2, LH], BF16, tag="pq2")
                                nc.vector.tensor_scalar_add(pq2[:], X, EC2)
                                nc.vector.tensor_tensor(pq2[:], pq2[:], pq2[:], OP.mult)
                                nc.vector.tensor_scalar_add(pq2[:], pq2[:], ED2)
                                nc.vector.tensor_tensor(X, pq1[:], pq2[:], OP.mult)

                        if act_exp:
                            for q in range(4):
                                nc.scalar.activation(
                                    expT[:, 4 * q:4 * (q + 1), :],
                                    expT[:, 4 * q:4 * (q + 1), :],
                                    AF.Exp, scale=INV_SQRT_2)

                        # PV matmul; row 64 of the result is the softmax denom
                        ot_stage = smal.tile([65, LH], F32, tag="ot_stage")
                        for lp in range(2):
                            pv = pspv.tile([65, 512], F32, tag="pv")
                            for sc in range(SC):
                                nc.tensor.matmul(
                                    pv[:],
                                    lhsT=vo_sb[:, sc, h, :],
                                    rhs=expT[:, sc, lp * 512:(lp + 1) * 512],
                                    start=(sc == 0), stop=(sc == SC - 1),
                                )
                            nc.vector.tensor_copy(ot_stage[:, lp * 512:(lp + 1) * 512], pv[:])

                        # reciprocal of denominator -> bf16 -> broadcast + transpose
                        se_row = smal.tile([1, LH], F32, tag="se_row")
                        nc.sync.dma_start(se_row[:], ot_stage[64:65, :])
                        nc.vector.reciprocal(se_row[:], se_row[:])
                        rec_b = smal.tile([1, LH], BF16, tag="rec_b")
                        nc.vector.tensor_copy(rec_b[:], se_row[:])
                        rec_bc = smal.tile([64, LH], BF16, tag="rec_bc")
                        nc.gpsimd.partition_broadcast(rec_bc[:], rec_b[:])
                        recT = smal.tile([P, 8, 16], BF16, tag="recT")
                        nc.sync.dma_start_transpose(recT[:], rec_bc[0:16, :])
                        recTf = smal.tile([P, 8, 1], F32, tag="recTf")
                        nc.vector.tensor_copy(recTf[:], recT[:, :, 0:1])

                        # normalize OT rows and park them for out_proj
                        normh = smal.tile([64, LH], BF16, tag="normh")
                        nc.vector.tensor_tensor(normh[:], ot_stage[0:64, :], rec_bc[:], OP.mult)
                        nc.sync.dma_start(OT_all[bp:bp + 64, dc, l0:l0 + LH], normh[:])

                        # transpose unnormalized exp^T -> [l,s], normalize per
                        # l-partition, one merged casting store per 512 l-rows
                        for gp in range(2):
                            tb = tbp.tile([P, 4, SC, P], BF16, tag="tb")
                            for sc in range(SC):
                                nc.sync.dma_start_transpose(
                                    tb[:, :, sc, :],
                                    expT[:, sc, gp * 512:(gp + 1) * 512],
                                )
                            for j in range(4):
                                nc.vector.tensor_scalar_mul(
                                    tb[:, j, :, :], tb[:, j, :, :],
                                    recTf[:, gp * 4 + j, :])
                            nc.gpsimd.dma_start(
                                attn_d[h, l0 + gp * 512:l0 + (gp + 1) * 512, :]
                                .rearrange("(j p) s -> p j s", p=P),
                                tb[:],
                            )

                emit_outproj(0, SC)

    nc.finalize()
    return nc


def _rope_tables():
    inv_freq = 1.0 / (10000.0 ** (np.arange(0, HD, 2, dtype=np.float32) / np.float32(HD)))
    freqs = np.arange(L, dtype=np.float32)[:, None] * inv_freq[None, :]   # [L, 32]
    emb = np.concatenate([freqs, freqs], axis=-1).astype(np.float32)      # [L, 64]
    cos = np.cos(emb).astype(np.float32)
    sin = np.sin(emb).astype(np.float32)
    d = np.arange(P) % HD
    cosT = cos[:, d].T.copy()                    # [128, L]
    sign = np.where((np.arange(P) % HD) < 32, 1.0, -1.0).astype(np.float32)
    sinT = (sin[:, d].T * sign[:, None]).copy()  # [128, L]
    bf16 = ml_dtypes.bfloat16
    return np.ascontiguousarray(cosT).astype(bf16), np.ascontiguousarray(sinT).astype(bf16)


def _make_in_maps(query, Wqw, Wkw, Wqp, Wkp, Wout):
    bf16 = ml_dtypes.bfloat16
    cosT, sinT = _rope_tables()
    in_maps = []
    for c in range(N_CORES):
        b, g = divmod(c, 4)
        rows = slice(DK * g, DK * (g + 1))
        qT = np.ascontiguousarray(query[b].T).astype(bf16)              # [1024, 2048]
        w = np.stack([Wqw[rows].T, Wkw[rows].T, Wqp[rows].T, Wkp[rows].T],
                     axis=1).astype(bf16)                               # [1024, 4, 256]
        v = query[b][:, rows]                                           # [2048, 256]
        vo = np.empty((L, NH, HD + 1), dtype=np.float32)
        vo[:, :, :HD] = v.reshape(L, NH, HD)
        vo[:, :, HD] = 1.0
        woutT = np.ascontiguousarray(Wout[:, rows].T).astype(bf16)      # [256, 1024]
        in_maps.append({
            "qT": qT,
            "w": np.ascontiguousarray(w),
            "vo": vo.astype(bf16),
            "woutT": woutT,
            "cosT": cosT,
            "sinT": sinT,
        })
    return in_maps


def run_all(query, mask, Wqw, Wkw, Wqp, Wkp, Wout, trace=False):
    if "nc" not in _CACHED:
        _CACHED["nc"] = _build_nc()
    nc = _CACHED["nc"]
    in_maps = _make_in_maps(query, Wqw, Wkw, Wqp, Wkp, Wout)
    res = run_bass_kernel_spmd(nc, in_maps, core_ids=list(range(N_CORES)), trace=trace)

    B = query.shape[0]
    attn = np.empty((B, 16, L, L), dtype=np.float32)
    out = np.zeros((B, L, DM), dtype=np.float32)
    for c in range(N_CORES):
        b, g = divmod(c, 4)
        attn[b, NH * g:NH * (g + 1)] = res.results[c]["attn"]
        out[b] += res.results[c]["part"]
    return (out, attn), res


def kernel(query, mask, Wqw, Wkw, Wqp, Wkp, Wout):
    query = np.asarray(query, dtype=np.float32)
    (out, attn), _ = run_all(
        query, np.asarray(mask),
        np.asarray(Wqw, dtype=np.float32), np.asarray(Wkw, dtype=np.float32),
        np.asarray(Wqp, dtype=np.float32), np.asarray(Wkp, dtype=np.float32),
        np.asarray(Wout, dtype=np.float32),
    )
    return out, attn


# revision 42
# speedup vs baseline: 1.0019x; 1.0019x over previous
"""MultiHeadCoupledAttention Trainium2 kernel (8 NeuronCores).

Sharding: core c handles batch b = c//4 and heads 4g..4g+3 where g = c%4
(data parallel on B, tensor parallel on heads / d_k; out_proj row-parallel
with the partial sums reduced on host).

Per-core device pipeline (all matmuls bf16, PSUM f32):
  1. QK projections (transposed layout [d_k, L]) evicted to bf16 on
     ScalarE, RoPE on VectorE with an in-SBUF half-swap DMA.
  2. Per (head, l-half): scores S^T[s,l] via K=64 matmuls; silu on
     ScalarE straight out of PSUM; add on VectorE; one big exp
     (scale=1/sqrt2) on ScalarE.  l-halving keeps the exp buffer
     double-buffered inside the SBUF budget.
  3. PV matmul with a ones-column appended to V: rows 0..63 = exp @ v,
     row 64 = softmax denominator.
  4. attn output: XBAR DMA-transpose of the unnormalized exp to [l,s],
     per-partition tensor_scalar normalize (reciprocal transposed via a
     second tiny XBAR transpose), casting SWDGE DMA store (bf16 -> f32).
  5. out_proj: OT_norm^T @ Wout_slice^T -> partial [L, 1024] f32.
"""

import math

import numpy as np
import ml_dtypes

import concourse.bass as bass
import concourse.mybir as mybir
import concourse.tile as tile
from concourse import bacc
from concourse.bass_utils import run_bass_kernel_spmd

BF16 = mybir.dt.bfloat16
F32 = mybir.dt.float32
AF = mybir.ActivationFunctionType
OP = mybir.AluOpType

P = 128
L = 2048          # sequence length
LH = 1024         # l-half width
DM = 1024         # d_model
NH = 4            # heads per core
HD = 64           # head dim
DK = NH * HD      # per-core d_k slice (256)
N_CORES = 8
SC = L // P       # 16 s-chunks

INV_SQRT_D = 1.0 / math.sqrt(HD)
INV_SQRT_2 = 1.0 / math.sqrt(2.0)

# exp(u/sqrt2) ~ C * ((u+C1)^2 + D1) * ((u+C2)^2 + D2) on u in [-0.45, 2.2]
# (C cancels in softmax).  Constants fitted against the exact bf16
# evaluation chain; max rel err ~0.97% incl. rounding.
EC1 = 1.58173174
ED1 = 1.27719784
EC2 = -0.69679504
ED2 = 10.81390516

_CACHED = {}


def _build_nc():
    nc = bacc.Bacc("TRN2", target_bir_lowering=False, debug=False)

    qT_d = nc.declare_dram_parameter("qT", [DM, L], BF16, isOutput=False)
    w_d = nc.declare_dram_parameter("w", [DM, 4, DK], BF16, isOutput=False)
    vo_d = nc.declare_dram_parameter("vo", [L, NH, HD + 1], BF16, isOutput=False)
    woutT_d = nc.declare_dram_parameter("woutT", [DK, DM], BF16, isOutput=False)
    cos_d = nc.declare_dram_parameter("cosT", [P, L], BF16, isOutput=False)
    sin_d = nc.declare_dram_parameter("sinT", [P, L], BF16, isOutput=False)
    attn_d = nc.declare_dram_parameter("attn", [NH, L, L], F32, isOutput=True)
    part_d = nc.declare_dram_parameter("part", [L, DM], F32, isOutput=True)

    with tile.TileContext(nc) as tc:
        with tc.tile_pool(name="const", bufs=1) as c0, \
             tc.tile_pool(name="proj", bufs=1) as projp:

            vo_sb = c0.tile([P, SC, NH, HD + 1], BF16)
            woutT_sb = c0.tile([P, 2, DM], BF16)

            # ---------------- Phase 1: projections + RoPE ----------------
            # QK[t] layout: [P, 2, L], dk row = dc*128 + p, t in (qw, kw, qp, kp)
            QK = [projp.tile([P, 2, L], BF16, name=f"qk{t}") for t in range(4)]
            with tc.tile_pool(name="ph1", bufs=1) as ph1, \
                 tc.tile_pool(name="ph1t", bufs=2) as ph1t, \
                 tc.tile_pool(name="ph1p", bufs=4, space="PSUM") as ph1p:
                qT_sb = ph1.tile([P, 8, L], BF16)
                w_sb = ph1.tile([P, 8, 4, DK], BF16)
                # chunked loads so the first projection matmuls start early
                for k in range(8):
                    nc.sync.dma_start(
                        qT_sb[:, k, :],
                        qT_d[:].rearrange("(ko p) l -> p ko l", p=P)[:, k, :])
                    nc.sync.dma_start(
                        w_sb[:, k, :, :],
                        w_d[:].rearrange("(ko p) t m -> p ko t m", p=P)[:, k, :, :])
                cos_sb = ph1.tile([P, L], BF16)
                nc.sync.dma_start(cos_sb[:], cos_d[:])
                sin_sb = ph1.tile([P, L], BF16)
                nc.sync.dma_start(sin_sb[:], sin_d[:])
                nc.sync.dma_start(
                    vo_sb[:], vo_d[:].rearrange("(sc p) h m -> p sc h m", p=P))
                nc.sync.dma_start(
                    woutT_sb[:], woutT_d[:].rearrange("(dc p) m -> p dc m", p=P))

                # dc-major so heads 0/1 (dc=0) unblock before dc=1 finishes;
                # qw/kp first so the w2p score matmuls can start earliest
                for dc in range(2):
                    for t in (0, 3, 1, 2):
                        xev = ph1t.tile([P, L], BF16, tag="xev")
                        for lc in range(4):
                            ps = ph1p.tile([P, 512], F32, tag="pp")
                            for k in range(8):
                                nc.tensor.matmul(
                                    ps[:],
                                    lhsT=w_sb[:, k, t, dc * 128:(dc + 1) * 128],
                                    rhs=qT_sb[:, k, lc * 512:(lc + 1) * 512],
                                    start=(k == 0), stop=(k == 7),
                                )
                            nc.scalar.activation(
                                xev[:, lc * 512:(lc + 1) * 512], ps[:], AF.Copy)
                        tcb = ph1t.tile([P, L], BF16, tag="tcb")
                        nc.vector.tensor_tensor(tcb[:], xev[:], cos_sb[:], OP.mult)
                        tsb = ph1t.tile([P, L], BF16, tag="tsb")
                        nc.vector.tensor_tensor(tsb[:], xev[:], sin_sb[:], OP.mult)
                        tswb = ph1t.tile([P, L], BF16, tag="tswb")
                        nc.sync.dma_start(tswb[0:32, :], tsb[32:64, :])
                        nc.sync.dma_start(tswb[32:64, :], tsb[0:32, :])
                        nc.sync.dma_start(tswb[64:96, :], tsb[96:128, :])
                        nc.sync.dma_start(tswb[96:128, :], tsb[64:96, :])
                        nc.vector.tensor_tensor(QK[t][:, dc, :], tcb[:], tswb[:], OP.add)

            OT_all = projp.tile([P, 2, L], BF16, name="ot_all")

            # ---------------- Phase 2: per (head, l-half) attention ----------------
            with tc.tile_pool(name="big", bufs=2) as bigp, \
                 tc.tile_pool(name="sct", bufs=4) as sct, \
                 tc.tile_pool(name="tbp", bufs=1) as tbp, \
                 tc.tile_pool(name="smal", bufs=2) as smal, \
                 tc.tile_pool(name="ps2", bufs=2, space="PSUM") as ps2:

                def emit_outproj(lc_lo, lc_hi):
                    for lc in range(lc_lo, lc_hi):
                        po = ps2.tile([P, 1024], F32, tag="scps")
                        for ns in range(2):
                            for dcq in range(2):
                                nc.tensor.matmul(
                                    po[:, ns * 512:(ns + 1) * 512],
                                    lhsT=OT_all[:, dcq, lc * P:(lc + 1) * P],
                                    rhs=woutT_sb[:, dcq, ns * 512:(ns + 1) * 512],
                                    start=(dcq == 0), stop=(dcq == 1),
                                )
                        osb = smal.tile([P, 1024], F32, tag="osb")
                        nc.any.tensor_copy(osb[:], po[:])
                        nc.sync.dma_start(part_d[lc * P:(lc + 1) * P, :], osb[:])

                for h in range(NH):
                    for half in range(2):
                        dc = h // 2
                        bp = 64 * (h % 2)
                        kpT = QK[3][bp:bp + 64, dc, :]
                        qwT = QK[0][bp:bp + 64, dc, :]
                        kwT = QK[1][bp:bp + 64, dc, :]
                        qpT = QK[2][bp:bp + 64, dc, :]
                        l0 = half * LH
                        expT = bigp.tile([P, SC, LH], BF16, tag="expT")
                        # ScalarE is the kernel's critical engine, so most
                        # halves evaluate exp as a factored quartic on
                        # VectorE (interleaved with the score adds so the
                        # DVE FIFO never starves ScalarE); a few halves stay
                        # on ScalarE to balance the two engines.
                        act_exp = True  # poly-exp path kept but disabled: scheduler loses more than engine balance gains

                        for sc in range(SC):
                            ssl = slice(sc * P, (sc + 1) * P)
                            psAB = ps2.tile([P, 2, LH], F32, tag="scps")
                            for lq in range(2):
                                q0 = l0 + lq * 512
                                nc.tensor.matmul(
                                    psAB[:, 0, lq * 512:(lq + 1) * 512],
                                    lhsT=kpT[:, ssl], rhs=qwT[:, q0:q0 + 512],
                                    start=True, stop=True,
                                )
                            for lq in range(2):
                                q0 = l0 + lq * 512
                                nc.tensor.matmul(
                                    psAB[:, 1, lq * 512:(lq + 1) * 512],
                                    lhsT=kwT[:, ssl], rhs=qpT[:, q0:q0 + 512],
                                    start=True, stop=True,
                                )
                            s12 = sct.tile([P, 2, LH], BF16, tag="s1")
                            nc.scalar.activation(s12[:], psAB[:], AF.Silu, scale=INV_SQRT_D)
                            nc.vector.tensor_tensor(expT[:, sc, :], s12[:, 0, :], s12[:, 1, :], OP.add)
                            if not act_exp and sc % 2 == 1:
                                X = expT[:, sc - 1:sc + 1, :]
                                pq1 = sct.tile([P, 2, LH], BF16, tag="pq1")
                                nc.vector.tensor_scalar_add(pq1[:], X, EC1)
                                nc.vector.tensor_tensor(pq1[:], pq1[:], pq1[:], OP.mult)
                                nc.vector.tensor_scalar_add(pq1[:], pq1[:], ED1)
                                pq2 = sct.tile([P, 2, LH], BF16, tag="pq2")
                                nc.vector.tensor_scalar_add(pq2[:], X, EC2)
                                nc.vector.tensor_tensor(pq2[:], pq2[:], pq2[:], OP.mult)
                                nc.vector.tensor_scalar_add(pq2[:], pq2[:], ED2)
                                nc.vector.tensor_tensor(X, pq1[:], pq2[:], OP.mult)

                        if act_exp:
                            for q in range(4):
                                nc.scalar.activation(
                                    expT[:, 4 * q:4 * (q + 1), :],
                                    expT[:, 4 * q:4 * (q + 1), :],
                                    AF.Exp, scale=INV_SQRT_2)

                        # PV matmul; row 64 of the result is the softmax denom
                        ot_stage = smal.tile([65, LH], F32, tag="ot_stage")
                        for lp in range(2):
                            pv = ps2.tile([65, 512], F32, tag="scps")
                            for sc in range(SC):
                                nc.tensor.matmul(
                                    pv[:],
                                    lhsT=vo_sb[:, sc, h, :],
                                    rhs=expT[:, sc, lp * 512:(lp + 1) * 512],
                                    start=(sc == 0), stop=(sc == SC - 1),
                                )
                            nc.vector.tensor_copy(ot_stage[:, lp * 512:(lp + 1) * 512], pv[:])

                        # reciprocal of denominator -> bf16 -> broadcast + transpose
                        se_row = smal.tile([1, LH], F32, tag="se_row")
                        nc.sync.dma_start(se_row[:], ot_stage[64:65, :])
                        nc.vector.reciprocal(se_row[:], se_row[:])
                        rec_b = smal.tile([1, LH], BF16, tag="rec_b")
                        nc.vector.tensor_copy(rec_b[:], se_row[:])
                        rec_bc = smal.tile([64, LH], BF16, tag="rec_bc")
                        nc.gpsimd.partition_broadcast(rec_bc[:], rec_b[:])
                        recT = smal.tile([P, 8, 16], BF16, tag="recT")
                        nc.sync.dma_start_transpose(recT[:], rec_bc[0:16, :])
                        recTf = smal.tile([P, 8, 1], F32, tag="recTf")
                        nc.vector.tensor_copy(recTf[:], recT[:, :, 0:1])

                        # normalize OT rows and park them for out_proj
                        normh = smal.tile([64, LH], BF16, tag="normh")
                        nc.vector.tensor_tensor(normh[:], ot_stage[0:64, :], rec_bc[:], OP.mult)
                        nc.sync.dma_start(OT_all[bp:bp + 64, dc, l0:l0 + LH], normh[:])

                        # transpose unnormalized exp^T -> [l,s], normalize per
                        # l-partition, one merged casting store per 512 l-rows
                        for gp in range(2):
                            tb = tbp.tile([P, 4, SC, P], BF16, tag="tb")
                            for sc in range(SC):
                                nc.sync.dma_start_transpose(
                                    tb[:, :, sc, :],
                                    expT[:, sc, gp * 512:(gp + 1) * 512],
                                )
                            for j in range(4):
                                nc.vector.tensor_scalar_mul(
                                    tb[:, j, :, :], tb[:, j, :, :],
                                    recTf[:, gp * 4 + j, :])
                            nc.gpsimd.dma_start(
                                attn_d[h, l0 + gp * 512:l0 + (gp + 1) * 512, :]
                                .rearrange("(j p) s -> p j s", p=P),
                                tb[:],
                            )

                emit_outproj(0, SC)

    nc.finalize()
    return nc


def _rope_tables():
    inv_freq = 1.0 / (10000.0 ** (np.arange(0, HD, 2, dtype=np.float32) / np.float32(HD)))
    freqs = np.arange(L, dtype=np.float32)[:, None] * inv_freq[None, :]   # [L, 32]
    emb = np.concatenate([freqs, freqs], axis=-1).astype(np.float32)      # [L, 64]
    cos = np.cos(emb).astype(np.float32)
    sin = np.sin(emb).astype(np.float32)
    d = np.arange(P) % HD
    cosT = cos[:, d].T.copy()                    # [128, L]
    sign = np.where((np.arange(P) % HD) < 32, 1.0, -1.0).astype(np.float32)
    sinT = (sin[:, d].T * sign[:, None]).copy()  # [128, L]
    bf16 = ml_dtypes.bfloat16
    return np.ascontiguousarray(cosT).astype(bf16), np.ascontiguousarray(sinT).astype(bf16)


def _make_in_maps(query, Wqw, Wkw, Wqp, Wkp, Wout):
    bf16 = ml_dtypes.bfloat16
    cosT, sinT = _rope_tables()
    in_maps = []
    for c in range(N_CORES):
        b, g = divmod(c, 4)
        rows = slice(DK * g, DK * (g + 1))
        qT = np.ascontiguousarray(query[b].T).astype(bf16)              # [1024, 2048]
        w = np.stack([Wqw[rows].T, Wkw[rows].T, Wqp[rows].T, Wkp[rows].T],
                     axis=1).astype(bf16)                               # [1024, 4, 256]
        v = query[b][:, rows]                                           # [2048, 256]
        vo = np.empty((L, NH, HD + 1), dtype=np.float32)
        vo[:, :, :HD] = v.reshape(L, NH, HD)
        vo[:, :, HD] = 1.0
        woutT = np.ascontiguousarray(Wout[:, rows].T).astype(bf16)      # [256, 1024]
        in_maps.append({
            "qT": qT,
            "w": np.ascontiguousarray(w),
            "vo": vo.astype(bf16),
            "woutT": woutT,
            "cosT": cosT,
            "sinT": sinT,
        })
    return in_maps


def run_all(query, mask, Wqw, Wkw, Wqp, Wkp, Wout, trace=False):
    if "nc" not in _CACHED:
        _CACHED["nc"] = _build_nc()
    nc = _CACHED["nc"]
    in_maps = _make_in_maps(query, Wqw, Wkw, Wqp, Wkp, Wout)
    res = run_bass_kernel_spmd(nc, in_maps, core_ids=list(range(N_CORES)), trace=trace)

    B = query.shape[0]
    attn = np.empty((B, 16, L, L), dtype=np.float32)
    out = np.zeros((B, L, DM), dtype=np.float32)
    for c in range(N_CORES):
        b, g = divmod(c, 4)
        attn[b, NH * g:NH * (g + 1)] = res.results[c]["attn"]
        out[b] += res.results[c]["part"]
    return (out, attn), res


def kernel(query, mask, Wqw, Wkw, Wqp, Wkp, Wout):
    query = np.asarray(query, dtype=np.float32)
    (out, attn), _ = run_all(
        query, np.asarray(mask),
        np.asarray(Wqw, dtype=np.float32), np.asarray(Wkw, dtype=np.float32),
        np.asarray(Wqp, dtype=np.float32), np.asarray(Wkp, dtype=np.float32),
        np.asarray(Wout, dtype=np.float32),
    )
    return out, attn
